# revision 11
# baseline (speedup 1.0000x reference)
"""Trainium2 Bass kernel for nn_L2GESRModule.

Reference computation:
    Fh_conv = Fh @ Wh + bh            (dead: only used via ones_like)
    ESF     = ones_like(Fh_conv)      -> gather indices are a fixed shift
    Y       = Fl @ Wl + bl
    out[b,i,j,:] = Y[b, min(i+1,H-1), min(j+1,W-1), :]

The whole problem is one 1x1-conv GEMM on Fl plus a static (+1,+1)
clamped-shift, data-parallel over batch (1 batch element per core). The
Fh/Wh/bh branch contributes nothing and is never loaded.

Flat-pixel layout: image = 16384 pixels; out[O] = Y[O + 129] except
col-127 cells (O%128==127) which need Y[O + 128] (clamped col), and the
last row which duplicates row H-2.

Per-core, chunks of 2048 pixels: SBUF tiles [128 parts, 16 slots, 256ch],
partition p = 16 *consecutive* pixels -> 16KB contiguous per partition ->
128 large DMA descriptors per 2MB transfer (HWDGE descriptor-gen bound
otherwise). Uniform chunk c loads src window [2048c+129, +2048) so every
compute group k writes ybig[:, k] unshifted. Col-127 cells are then fixed
by copying slot (p=7 mod 8, k=14) -> (p, 15) with a tiny strided SBUF->SBUF
DMA (engines cannot partition-shift; DMA can). The last chunk's window
would run off the input, so it loads [P-1920, P) (+128-style), shifting
group 0's result by one partition via a small SBUF->SBUF DMA.

Compute per 128-pixel group: 2x PE transpose (fp32) -> PSUM -> ACT evac to
SBUF as X^T (cast to fp32r) -> 2x PE matmul (fp32r, full rate) accumulate
-> DVE adds bias PSUM->SBUF.
"""

import numpy as np

import concourse.bacc as bacc
import concourse.mybir as mybir
from concourse import bass_utils, tile
from concourse.masks import make_identity

B, H, W, CIN, COUT = 8, 128, 128, 256, 256
N_CORES = 8
MM_DT = mybir.dt.float32r  # fp32r: full-rate PE, ~19-bit mantissa products


def build_nc(n_rows: int = H, mm_dt=MM_DT):
    f32 = mybir.dt.float32
    P = n_rows * W  # total pixels per image
    assert P % 2048 == 0 and P >= 2048
    n_chunks = P // 2048

    nc = bacc.Bacc("TRN2", target_bir_lowering=False, debug=False)
    Fl = nc.dram_tensor("Fl", [P, CIN], f32, kind="ExternalInput").ap()
    Wl = nc.dram_tensor("Wl", [CIN, COUT], f32, kind="ExternalInput").ap()
    bl = nc.dram_tensor("bl", [COUT], f32, kind="ExternalInput").ap()
    # mask over partitions (1.0 on p % 8 == 7): engines cannot address
    # strided partitions, so the col-127 patch is a masked predicated copy
    msk = nc.dram_tensor("msk", [128, COUT], mybir.dt.uint8, kind="ExternalInput").ap()
    out = nc.dram_tensor("out", [P, COUT], f32, kind="ExternalOutput").ap()

    with tile.TileContext(nc) as tc:
        with (
            tc.tile_pool(name="consts", bufs=1) as consts,
            tc.tile_pool(name="xin", bufs=3) as xin_pool,
            tc.tile_pool(name="xt", bufs=3) as xt_pool,
            tc.tile_pool(name="yout", bufs=2) as yout_pool,
            tc.tile_pool(name="tmp", bufs=1) as tmp_pool,
            tc.tile_pool(name="pt", bufs=3, space="PSUM") as pt_pool,
            tc.tile_pool(name="py", bufs=3, space="PSUM") as py_pool,
            tc.tile_pool(name="pb", bufs=1, space="PSUM") as pb_pool,
        ):
            ident = consts.tile([128, 128], f32)
            make_identity(nc, ident)

            # Wl as two K-chunks: w_sb[c, kc, n] = Wl[kc*128 + c, n].
            # fp32r matmul operands must be rounded to fp32r by their
            # producer, so cast during the DMA (SWDGE).
            w_sb = consts.tile([128, 2, COUT], mm_dt)
            w_src = Wl.rearrange("(kc kp) n -> kp kc n", kp=128)
            if mm_dt == f32:
                nc.sync.dma_start(w_sb, w_src)
            else:
                nc.gpsimd.dma_start(w_sb, w_src)

            # bias broadcast to all 128 partitions via ones[128,1] @ bl[1,256]
            ones = consts.tile([1, 128], f32)
            nc.gpsimd.memset(ones, 1.0)
            bl_sb = consts.tile([1, COUT], f32)
            nc.sync.dma_start(bl_sb, bl[None, :])
            bias_ps = pb_pool.tile([128, COUT], f32)
            nc.tensor.matmul(bias_ps, ones, bl_sb, start=True, stop=True)
            bias_sb = consts.tile([128, COUT], f32)
            nc.scalar.copy(bias_sb, bias_ps)

            msk_sb = consts.tile([128, COUT], mybir.dt.uint8)
            nc.sync.dma_start(msk_sb, msk)

            def conv_group(x_slice, py_out, npart):
                """py_out[0:npart, :] = x_slice @ Wl   (x_slice: [npart, 256])"""
                pt = pt_pool.tile([128, 2, 128], f32, tag="pt")
                nc.tensor.transpose(pt[:, 0, :npart], x_slice[:, 0:128], ident[:npart, :npart])
                nc.tensor.transpose(pt[:, 1, :npart], x_slice[:, 128:256], ident[:npart, :npart])
                xt = xt_pool.tile([128, 2, 128], mm_dt, tag="xt")
                nc.scalar.copy(xt[:, :, :npart], pt[:, :, :npart])
                nc.tensor.matmul(py_out, xt[:, 0, :npart], w_sb[:, 0], start=True, stop=False)
                nc.tensor.matmul(py_out, xt[:, 1, :npart], w_sb[:, 1], start=False, stop=True)

            # ---- uniform chunks: out [2048c, +2048), src window +129 ----
            for c in range(n_chunks - 1):
                O0 = 2048 * c
                xbig = xin_pool.tile([128, 16, CIN], f32, tag="xin")
                nc.sync.dma_start(
                    xbig, Fl[O0 + 129 : O0 + 129 + 2048].rearrange("(p k) c -> p k c", k=16)
                )
                ybig = yout_pool.tile([128, 16, COUT], f32, tag="yout")
                for k in range(16):
                    py = py_pool.tile([128, COUT], f32, tag="py")
                    conv_group(xbig[:, k], py, 128)
                    nc.vector.tensor_add(ybig[:, k], py, bias_sb)
                # col-127 cells (slot k=15 on partitions 7 mod 8) duplicate
                # the col-126 value (slot k=14): masked predicated copy
                nc.vector.copy_predicated(ybig[:, 15], msk_sb, ybig[:, 14])
                nc.scalar.dma_start(
                    out[O0 : O0 + 2048].rearrange("(p k) c -> p k c", k=16), ybig
                )

            # ---- last chunk: out [P-2048, P-128) + duplicated final row ----
            O0 = P - 2048
            W0 = P - 1920  # src window [W0, P), 120 partitions
            xbig = xin_pool.tile([128, 16, CIN], f32, tag="xin")
            nc.sync.dma_start(
                xbig[0:120], Fl[W0:P].rearrange("(p k) c -> p k c", k=16)
            )
            ybig = yout_pool.tile([128, 16, COUT], f32, tag="yout")
            tmp0 = tmp_pool.tile([128, COUT], f32)
            for k in range(16):
                py = py_pool.tile([128, COUT], f32, tag="py")
                conv_group(xbig[0:120, k], py[0:120], 120)
                if k == 0:
                    # slot target is (p-1, 15): shift by one partition via DMA
                    nc.vector.tensor_add(tmp0[0:120], py[0:120], bias_sb[0:120])
                else:
                    nc.vector.tensor_add(ybig[0:120, k - 1], py[0:120], bias_sb[0:120])
            nc.sync.dma_start(ybig[0:119, 15], tmp0[1:120])
            nc.vector.copy_predicated(ybig[0:120, 15], msk_sb[0:120], ybig[0:120, 14])
            nc.scalar.dma_start(
                out[O0 : P - 128].rearrange("(p k) c -> p k c", k=16), ybig[0:120]
            )
            # final row (n_rows-1) = copy of row n_rows-2 (slots 1792..1919)
            nc.scalar.dma_start(
                out[P - 128 : P].rearrange("(p k) c -> p k c", k=16), ybig[112:120]
            )

    nc.compile()
    return nc


_cache: dict = {}


def _get_nc():
    if "nc" not in _cache:
        _cache["nc"] = build_nc()
    return _cache["nc"]


def make_mask():
    m = np.zeros((128, COUT), dtype=np.uint8)
    m[7::8, :] = 1
    return m


def kernel(Fh, Fl, Wh, bh, Wl, bl):
    nc = _get_nc()
    Fl = np.asarray(Fl, dtype=np.float32)
    Wl_np = np.ascontiguousarray(np.asarray(Wl, dtype=np.float32))
    bl_np = np.ascontiguousarray(np.asarray(bl, dtype=np.float32))
    msk_np = make_mask()
    in_maps = [
        {
            "Fl": np.ascontiguousarray(Fl[b].reshape(H * W, CIN)),
            "Wl": Wl_np,
            "bl": bl_np,
            "msk": msk_np,
        }
        for b in range(B)
    ]
    res = bass_utils.run_bass_kernel_spmd(nc, in_maps, core_ids=list(range(N_CORES)))
    return np.stack(
        [res.results[b]["out"].reshape(H, W, COUT) for b in range(B)], axis=0
    )


# revision 12
# speedup vs baseline: 1.1479x; 1.1479x over previous
"""Trainium2 Bass kernel for nn_L2GESRModule.

Reference computation:
    Fh_conv = Fh @ Wh + bh            (dead: only used via ones_like)
    ESF     = ones_like(Fh_conv)      -> gather indices are a fixed shift
    Y       = Fl @ Wl + bl
    out[b,i,j,:] = Y[b, min(i+1,H-1), min(j+1,W-1), :]

The whole problem is one 1x1-conv GEMM on Fl plus a static (+1,+1)
clamped-shift, data-parallel over batch (1 batch element per core). The
Fh/Wh/bh branch contributes nothing and is never loaded.

Flat-pixel layout: image = 16384 pixels; out[O] = Y[O + 129] except
col-127 cells (O%128==127) which need Y[O + 128] (clamped col), and the
last row which duplicates row H-2.

Per-core, chunks of 2048 pixels: SBUF tiles [128 parts, 16 slots, 256ch],
partition p = 16 *consecutive* pixels -> 16KB contiguous per partition ->
128 large DMA descriptors per 2MB transfer (HWDGE descriptor-gen bound
otherwise). Uniform chunk c loads src window [2048c+129, +2048) so every
compute group k writes ybig[:, k] unshifted. Col-127 cells are then fixed
by copying slot (p=7 mod 8, k=14) -> (p, 15) with a tiny strided SBUF->SBUF
DMA (engines cannot partition-shift; DMA can). The last chunk's window
would run off the input, so it loads [P-1920, P) (+128-style), shifting
group 0's result by one partition via a small SBUF->SBUF DMA.

Compute per 128-pixel group: 2x PE transpose (fp32) -> PSUM -> ACT evac to
SBUF as X^T (cast to fp32r) -> 2x PE matmul (fp32r, full rate) accumulate
-> DVE adds bias PSUM->SBUF.
"""

import numpy as np

import concourse.bacc as bacc
import concourse.mybir as mybir
from concourse import bass_utils, tile
from concourse.masks import make_identity

B, H, W, CIN, COUT = 8, 128, 128, 256, 256
N_CORES = 8
MM_DT = mybir.dt.float32r  # fp32r: full-rate PE, ~19-bit mantissa products


def build_nc(n_rows: int = H, mm_dt=MM_DT):
    f32 = mybir.dt.float32
    P = n_rows * W  # total pixels per image
    assert P % 2048 == 0 and P >= 2048
    n_chunks = P // 2048

    nc = bacc.Bacc("TRN2", target_bir_lowering=False, debug=False)
    Fl = nc.dram_tensor("Fl", [P, CIN], f32, kind="ExternalInput").ap()
    Wl = nc.dram_tensor("Wl", [CIN, COUT], f32, kind="ExternalInput").ap()
    bl = nc.dram_tensor("bl", [COUT], f32, kind="ExternalInput").ap()
    # mask over partitions (1.0 on p % 8 == 7): engines cannot address
    # strided partitions, so the col-127 patch is a masked predicated copy
    msk = nc.dram_tensor("msk", [128, COUT], mybir.dt.uint8, kind="ExternalInput").ap()
    out = nc.dram_tensor("out", [P, COUT], f32, kind="ExternalOutput").ap()

    with tile.TileContext(nc) as tc:
        with (
            tc.tile_pool(name="consts", bufs=1) as consts,
            tc.tile_pool(name="xin", bufs=3) as xin_pool,
            tc.tile_pool(name="xt", bufs=3) as xt_pool,
            tc.tile_pool(name="yout", bufs=3) as yout_pool,
            tc.tile_pool(name="tmp", bufs=1) as tmp_pool,
            tc.tile_pool(name="pt", bufs=3, space="PSUM") as pt_pool,
            tc.tile_pool(name="py", bufs=4, space="PSUM") as py_pool,
            tc.tile_pool(name="pb", bufs=1, space="PSUM") as pb_pool,
        ):
            ident = consts.tile([128, 128], f32)
            make_identity(nc, ident)

            # Wl as two K-chunks: w_sb[c, kc, n] = Wl[kc*128 + c, n].
            # fp32r matmul operands must be rounded to fp32r by their
            # producer, so cast during the DMA (SWDGE).
            w_sb = consts.tile([128, 2, COUT], mm_dt)
            w_src = Wl.rearrange("(kc kp) n -> kp kc n", kp=128)
            if mm_dt == f32:
                nc.sync.dma_start(w_sb, w_src)
            else:
                nc.gpsimd.dma_start(w_sb, w_src)

            # bias broadcast to all 128 partitions via ones[128,1] @ bl[1,256]
            ones = consts.tile([1, 128], f32)
            nc.gpsimd.memset(ones, 1.0)
            bl_sb = consts.tile([1, COUT], f32)
            nc.sync.dma_start(bl_sb, bl[None, :])
            bias_ps = pb_pool.tile([128, COUT], f32)
            nc.tensor.matmul(bias_ps, ones, bl_sb, start=True, stop=True)
            bias_sb = consts.tile([128, COUT], f32)
            nc.scalar.copy(bias_sb, bias_ps)

            msk_sb = consts.tile([128, COUT], mybir.dt.uint8)
            nc.sync.dma_start(msk_sb, msk)

            def conv_group(x_slice, py_out, npart):
                """py_out[0:npart, :] = x_slice @ Wl   (x_slice: [npart, 256])"""
                pt = pt_pool.tile([128, 2, 128], f32, tag="pt")
                nc.tensor.transpose(pt[:, 0, :npart], x_slice[:, 0:128], ident[:npart, :npart])
                nc.tensor.transpose(pt[:, 1, :npart], x_slice[:, 128:256], ident[:npart, :npart])
                xt = xt_pool.tile([128, 2, 128], mm_dt, tag="xt")
                nc.scalar.copy(xt[:, :, :npart], pt[:, :, :npart])
                nc.tensor.matmul(py_out, xt[:, 0, :npart], w_sb[:, 0], start=True, stop=False)
                nc.tensor.matmul(py_out, xt[:, 1, :npart], w_sb[:, 1], start=False, stop=True)

            # ---- uniform chunks: out [2048c, +2048), src window +129 ----
            for c in range(n_chunks - 1):
                O0 = 2048 * c
                xbig = xin_pool.tile([128, 16, CIN], f32, tag="xin")
                src_w = Fl[O0 + 129 : O0 + 129 + 2048].rearrange("(p k) c -> p k c", k=16)
                nc.sync.dma_start(xbig[:, 0:8], src_w[:, 0:8])
                nc.sync.dma_start(xbig[:, 8:16], src_w[:, 8:16])
                ybig = yout_pool.tile([128, 16, COUT], f32, tag="yout")
                dst_w = out[O0 : O0 + 2048].rearrange("(p k) c -> p k c", k=16)
                for k in range(16):
                    py = py_pool.tile([128, COUT], f32, tag="py")
                    conv_group(xbig[:, k], py, 128)
                    nc.vector.tensor_add(ybig[:, k], py, bias_sb)
                    if k == 7:
                        nc.scalar.dma_start(dst_w[:, 0:8], ybig[:, 0:8])
                # col-127 cells (slot k=15 on partitions 7 mod 8) duplicate
                # the col-126 value (slot k=14): masked predicated copy
                nc.vector.copy_predicated(ybig[:, 15], msk_sb, ybig[:, 14])
                nc.scalar.dma_start(dst_w[:, 8:16], ybig[:, 8:16])

            # ---- last chunk: out [P-2048, P-128) + duplicated final row ----
            O0 = P - 2048
            W0 = P - 1920  # src window [W0, P), 120 partitions
            xbig = xin_pool.tile([128, 16, CIN], f32, tag="xin")
            nc.sync.dma_start(
                xbig[0:120], Fl[W0:P].rearrange("(p k) c -> p k c", k=16)
            )
            ybig = yout_pool.tile([128, 16, COUT], f32, tag="yout")
            tmp0 = tmp_pool.tile([128, COUT], f32)
            for k in range(16):
                py = py_pool.tile([128, COUT], f32, tag="py")
                conv_group(xbig[0:120, k], py[0:120], 120)
                if k == 0:
                    # slot target is (p-1, 15): shift by one partition via DMA
                    nc.vector.tensor_add(tmp0[0:120], py[0:120], bias_sb[0:120])
                else:
                    nc.vector.tensor_add(ybig[0:120, k - 1], py[0:120], bias_sb[0:120])
            nc.sync.dma_start(ybig[0:119, 15], tmp0[1:120])
            nc.vector.copy_predicated(ybig[0:120, 15], msk_sb[0:120], ybig[0:120, 14])
            nc.scalar.dma_start(
                out[O0 : P - 128].rearrange("(p k) c -> p k c", k=16), ybig[0:120]
            )
            # final row (n_rows-1) = copy of row n_rows-2 (slots 1792..1919)
            nc.scalar.dma_start(
                out[P - 128 : P].rearrange("(p k) c -> p k c", k=16), ybig[112:120]
            )

    nc.compile()
    return nc


_cache: dict = {}


def _get_nc():
    if "nc" not in _cache:
        _cache["nc"] = build_nc()
    return _cache["nc"]


def make_mask():
    m = np.zeros((128, COUT), dtype=np.uint8)
    m[7::8, :] = 1
    return m


def kernel(Fh, Fl, Wh, bh, Wl, bl):
    nc = _get_nc()
    Fl = np.asarray(Fl, dtype=np.float32)
    Wl_np = np.ascontiguousarray(np.asarray(Wl, dtype=np.float32))
    bl_np = np.ascontiguousarray(np.asarray(bl, dtype=np.float32))
    msk_np = make_mask()
    in_maps = [
        {
            "Fl": np.ascontiguousarray(Fl[b].reshape(H * W, CIN)),
            "Wl": Wl_np,
            "bl": bl_np,
            "msk": msk_np,
        }
        for b in range(B)
    ]
    res = bass_utils.run_bass_kernel_spmd(nc, in_maps, core_ids=list(range(N_CORES)))
    return np.stack(
        [res.results[b]["out"].reshape(H, W, COUT) for b in range(B)], axis=0
    )
